# revision 7
# baseline (speedup 1.0000x reference)
"""BlockSparseRingMultiheadDilatedAttention Trainium2 kernel (v2).

Problem (hardcoded): B=1, N=8192, E=1024, H=16 heads, D=64.
Two dilated groups: g0 = heads 0-7, seg 2048, dilation 1;
                    g1 = heads 8-15, seg 4096, dilation 2, offset 1 (odd positions).
Causal within each (gathered) segment.

Sharding over 8 cores (uniform SPMD program, per-core data):
  core c: a = c%2, sc = c//2, b = c%4, rc = c//4
    g0: seg sc (rows 2048*sc .. +2048), heads 4a..4a+4   (4 blocks of [2048 x 2048])
    g1: seg rc odd rows (gathered, 2048 rows), heads 8+2b..+2 (2 blocks)
  Host pre-slices inputs (bf16 cast, odd-row gather, weight head slices) so the
  device program is identical on every core.  Host sums the per-core partial
  output projections (disjoint head contributions) and adds bo + the bv
  pass-through term (softmax rows sum to 1, so bv adds linearly after).

v2 device dataflow per core (vs v1 baseline):
  - S^T score matmuls for the 2 heads of a pair are row-tiled (K=64 at
    tile_position (0,0)/(64,0)) and emitted adjacently -> run concurrently.
    s tile [128,1024] = kpos chunk i for BOTH heads (bank-aligned halves).
  - Causal trimming: diagonal chunks only compute/exp/mask columns >= trim.
  - One exp per chunk over both heads via a [128,2,W] strided AP.
  - Single triangle mask constant, applied to the 4 diagonal chunks per
    (j, pair) with a DVE multiply.
  - V projected directly in natural layout (x chunk as stationary operand),
    no PE transposes / per-chunk DVE copies.
  - reciprocal_approx_fast for softmax denominators (5x faster than
    nc.vector.reciprocal).
  - y0/y1 stored bf16 (halves output DMA).
  - Emission interleaves pair-2 projections and output projections into the
    ACT-paced attention phases so the PE stays busy.
"""

import numpy as np
import ml_dtypes

BF16 = ml_dtypes.bfloat16

SEG = 2048          # rows per attention block (both groups, post-gather)
E = 1024            # embedding
NQ = 512            # tq chunk (one PSUM bank of fp32)
NTQ = SEG // NQ     # 4 tq chunks per block
NTK = SEG // 128    # 16 tk chunks per block
ECH = E // 128      # 8 embedding chunks
VSP = 66            # per-head stride in the vn tile (64 v dims + ones + pad)

_CACHE = {}


def _build_program():
    import concourse.bacc as bacc
    import concourse.mybir as mybir
    import concourse.tile as tile

    dt = mybir.dt
    nc = bacc.Bacc("TRN2", target_bir_lowering=False, debug=False,
                   enable_asserts=False)

    # ---- DRAM I/O (uniform across cores; host slices per core) ----
    xs = {}
    for sel in ("a", "b"):      # a = g0 rows, b = g1 gathered odd rows
        for inp in ("q", "k", "v"):
            xs[(sel, inp)] = nc.dram_tensor(
                f"x{sel}_{inp}", [E, SEG], dt.bfloat16, kind="ExternalInput").ap()
    ws = {inp: nc.dram_tensor(f"w{inp}", [E, 384], dt.bfloat16,
                              kind="ExternalInput").ap()
          for inp in ("q", "k", "v")}
    wo = nc.dram_tensor("wo", [384, E], dt.bfloat16, kind="ExternalInput").ap()
    bs = {inp: nc.dram_tensor(f"b{inp}", [384, 1], dt.float32,
                              kind="ExternalInput").ap()
          for inp in ("q", "k")}
    y0 = nc.dram_tensor("y0", [SEG, E], dt.bfloat16, kind="ExternalOutput").ap()
    y1 = nc.dram_tensor("y1", [SEG, E], dt.bfloat16, kind="ExternalOutput").ap()

    with tile.TileContext(nc) as tc:
        from contextlib import ExitStack
        with ExitStack() as ctx:
            const = ctx.enter_context(tc.tile_pool(name="const", bufs=1))
            wpool = ctx.enter_context(tc.tile_pool(name="wpool", bufs=1))
            xtp = ctx.enter_context(tc.tile_pool(name="xtp", bufs=2))
            qkt = ctx.enter_context(tc.tile_pool(name="qkt", bufs=1))
            vnat = ctx.enter_context(tc.tile_pool(name="vnat", bufs=1))
            otp = ctx.enter_context(tc.tile_pool(name="otp", bufs=1))
            ptp = ctx.enter_context(tc.tile_pool(name="ptp", bufs=3))
            smallp = ctx.enter_context(tc.tile_pool(name="smallp", bufs=4))
            ypool = ctx.enter_context(tc.tile_pool(name="ypool", bufs=4))
            ps_mm = ctx.enter_context(
                tc.tile_pool(name="ps_mm", bufs=3, space="PSUM"))
            ps_acc = ctx.enter_context(
                tc.tile_pool(name="ps_acc", bufs=2, space="PSUM"))

            # ---- constants: triangle mask (shared by both heads/halves) ----
            # maskC[p, 512*h + w] = 1.0 if w >= p else 0.0
            maskC = const.tile([128, 2 * NQ], dt.bfloat16, tag="maskC")
            nc.gpsimd.memset(maskC, 1.0)
            nc.gpsimd.affine_select(
                out=maskC.rearrange("p (h w) -> p h w", h=2),
                in_=maskC.rearrange("p (h w) -> p h w", h=2),
                compare_op=mybir.AluOpType.is_ge,
                fill=0.0, base=0, pattern=[[0, 2], [1, NQ]],
                channel_multiplier=-1)

            # ---- weights (host-pre-transposed; plain DMA loads) ----
            # wTa[inp]: [128 e, ECH*384]; chunk ec pair p at 384*ec + 128*p
            wTa = {}
            for inp in ("q", "k", "v"):
                t = wpool.tile([128, 384 * ECH], dt.bfloat16,
                               tag=f"wT_{inp}", name=f"wT_{inp}")
                nc.sync.dma_start(
                    out=t.rearrange("p (ec m) -> p ec m", m=384),
                    in_=ws[inp].rearrange("(ec p) m -> p ec m", p=128))
                wTa[inp] = t
            # wT[inp][p]: 3D view [128 e, ECH, 128 d]; chunk ec = [:, ec, :]
            wT = {inp: [wTa[inp].rearrange("p (ec x) -> p ec x", x=384)
                        [:, :, 128 * p:128 * (p + 1)]
                        for p in range(3)] for inp in ("q", "k", "v")}
            # woT[p]: [128 (2 heads d), 1024 j]
            woT = []
            for p in range(3):
                t = wpool.tile([128, E], dt.bfloat16, tag=f"woT_{p}")
                nc.sync.dma_start(out=t, in_=wo[128 * p:128 * (p + 1), :])
                woT.append(t)
            # biases -> SBUF [128,1] per (inp, pair), q/k only
            bsb = {}
            for inp in ("q", "k"):
                for p in range(3):
                    t = wpool.tile([128, 1], dt.float32, tag=f"b_{inp}_{p}")
                    nc.sync.dma_start(
                        out=t, in_=bs[inp][128 * p:128 * (p + 1), :])
                    bsb[(inp, p)] = t

            # ---- persistent per-pair activations ----
            qT = [qkt.tile([128, SEG], dt.bfloat16, tag=f"qT{p}", name=f"qT{p}")
                  for p in range(3)]
            kT = [qkt.tile([128, SEG], dt.bfloat16, tag=f"kT{p}", name=f"kT{p}")
                  for p in range(3)]
            # V natural: per pair [128, NTK*2*VSP]; chunk i head h lhsT (65
            # cols: 64 v dims + ones) = [:, 2*VSP*i + VSP*h : +65]
            vn = [vnat.tile([128, NTK * 2 * VSP], dt.bfloat16,
                            tag=f"vn{p}", name=f"vn{p}")
                  for p in range(3)]
            oT = [otp.tile([128, SEG], dt.bfloat16, tag=f"oT{p}", name=f"oT{p}")
                  for p in range(3)]

            # ---- projection emitters (generators yielding PE quanta) ----
            def load_xt(sel, inp):
                xt = xtp.tile([128, ECH * SEG], dt.bfloat16, tag="xt")
                nc.sync.dma_start(
                    out=xt.rearrange("p (ec n) -> p ec n", n=SEG),
                    in_=xs[(sel, inp)].rearrange("(ec p) n -> p ec n", p=128))
                return xt

            def proj_qk(xt, inp, p):
                """Emit projection of one input tensor into qT/kT[p]."""
                dst = (qT if inp == "q" else kT)[p]
                for t2 in range(NTQ // 2):
                    acc = ps_mm.tile([128, 2 * NQ], dt.float32, tag="mm")
                    for half in range(2):
                        for ec in range(ECH):
                            nc.tensor.matmul(
                                acc[:, NQ * half:NQ * (half + 1)],
                                wT[inp][p][:, ec, :],
                                xt[:, SEG * ec + NQ * (2 * t2 + half):
                                   SEG * ec + NQ * (2 * t2 + half + 1)],
                                start=(ec == 0), stop=(ec == ECH - 1))
                        yield
                    nc.scalar.activation(
                        dst[:, 2 * NQ * t2:2 * NQ * (t2 + 1)], acc,
                        mybir.ActivationFunctionType.Identity,
                        bias=bsb[(inp, p)], scale=1.0)

            def proj_v(xt, p):
                """Emit V-natural projection into vn[p] (+ ones columns)."""
                # rank-3 views only (keep APs simple for lowering)
                vv = vn[p].rearrange("p (kh y) -> p kh y", y=VSP)
                nc.gpsimd.memset(vv[:, :, 64:65], 1.0)
                vk = vn[p].rearrange("p (k y) -> p k y", y=2 * VSP)
                for g in range(2):          # 8 kpos chunks per accv tile
                    accv = ps_mm.tile([128, 2 * NQ], dt.float32, tag="mm")
                    for kbs in range(8):
                        kb = 8 * g + kbs
                        for ec in range(ECH):
                            nc.tensor.matmul(
                                accv[:, 128 * kbs:128 * (kbs + 1)],
                                xt[:, SEG * ec + 128 * kb:
                                   SEG * ec + 128 * (kb + 1)],
                                wT["v"][p][:, ec, :],
                                start=(ec == 0), stop=(ec == ECH - 1))
                        if kbs % 2 == 1:
                            yield
                    av = accv.rearrange("p (k y) -> p k y", y=128)
                    for h in range(2):
                        nc.vector.tensor_copy(
                            vk[:, 8 * g:8 * (g + 1),
                               VSP * h:VSP * h + 64],
                            av[:, :, 64 * h:64 * h + 64])
                    yield

            def out_proj(ydram, pairs, ms):
                """Emit output-projection chunks for row-chunks in ms."""
                for m in ms:
                    accy = ps_mm.tile([128, 2 * NQ], dt.float32, tag="mm",
                                      name="accy")
                    for jc in range(2):
                        for idx, p in enumerate(pairs):
                            nc.tensor.matmul(
                                accy[:, NQ * jc:NQ * (jc + 1)],
                                oT[p][:, 128 * m:128 * (m + 1)],
                                woT[p][:, NQ * jc:NQ * (jc + 1)],
                                start=(idx == 0), stop=(idx == len(pairs) - 1))
                    ysb = ypool.tile([128, 2 * NQ], dt.bfloat16, tag="ysb")
                    nc.vector.tensor_copy(ysb, accy)
                    nc.sync.dma_start(
                        out=ydram[128 * m:128 * (m + 1), :], in_=ysb)
                    yield

            def drain(gen):
                if gen is not None:
                    for _ in gen:
                        pass

            # ---- attention for one pair, with filler interleaving ----
            def attention_pair(p, filler=None, post_j=None):
                def pull():
                    if filler is not None:
                        next(filler, None)

                for j in range(NTQ):
                    nchunks = 4 * (j + 1)
                    acc = [ps_acc.tile([128, NQ], dt.float32, tag="acc",
                                       name=f"acc{h}") for h in range(2)]
                    pend = []   # (i, trim, pt) awaiting O emission

                    def s_chunk(i):
                        """Row-tiled S matmuls + exp (+ mask) for chunk i."""
                        trim = max(0, 128 * i - NQ * j)
                        s = ps_mm.tile([128, 2 * NQ], dt.float32,
                                       tag="mm", name="s")
                        for h in range(2):
                            hp = 64 * h
                            nc.tensor.matmul(
                                s[:, NQ * h + trim:NQ * (h + 1)],
                                kT[p][hp:hp + 64, 128 * i:128 * (i + 1)],
                                qT[p][hp:hp + 64,
                                      NQ * j + trim:NQ * (j + 1)],
                                start=True, stop=True)
                        pt = ptp.tile([128, 2 * NQ], dt.bfloat16,
                                      tag="pt", name="pt")
                        sv = s.rearrange("p (h w) -> p h w", h=2)
                        pv = pt.rearrange("p (h w) -> p h w", h=2)
                        nc.scalar.activation(
                            pv[:, :, trim:NQ], sv[:, :, trim:NQ],
                            mybir.ActivationFunctionType.Exp,
                            bias=0.0, scale=0.125)
                        if trim > 0 or i == 4 * j:   # diagonal chunk
                            mv = maskC.rearrange("p (h w) -> p h w", h=2)
                            nc.vector.tensor_mul(
                                pv[:, :, trim:NQ], pv[:, :, trim:NQ],
                                mv[:, :, 0:NQ - trim])
                        return trim, pt

                    def o_chunk(i, trim, pt):
                        pv = pt.rearrange("p (h w) -> p h w", h=2)
                        for h in range(2):
                            nc.tensor.matmul(
                                acc[h][0:65, trim:NQ],
                                vn[p][:, 2 * VSP * i + VSP * h:
                                      2 * VSP * i + VSP * h + 65],
                                pv[:, h, trim:NQ],
                                start=(i == 0), stop=(i == nchunks - 1),
                                skip_group_check=True)

                    for i in range(nchunks):
                        pend.append((i, *s_chunk(i)))
                        if len(pend) > 1:
                            o_chunk(*pend.pop(0))
                            pull()
                    o_chunk(*pend.pop(0))
                    pull()

                    # normalize: r = 1/d, broadcast, multiply into oT
                    for h in range(2):
                        hp = 64 * h
                        rj = smallp.tile([1, NQ], dt.float32, tag="rj",
                                         name="rj")
                        nc.vector.reciprocal(rj, acc[h][64:65, :])
                        rb = smallp.tile([64, NQ], dt.float32, tag="rb",
                                         name="rb")
                        nc.gpsimd.partition_broadcast(rb, rj)
                        nc.vector.tensor_mul(
                            oT[p][hp:hp + 64, NQ * j:NQ * (j + 1)],
                            acc[h][0:64, :], rb)
                    if post_j is not None:
                        post_j(j)
                    pull()

            # ================= emission schedule =================
            # Head: load + project xa (pairs 0,1) densely.
            for inp in ("q", "k", "v"):
                xt = load_xt("a", inp)
                for p in (0, 1):
                    if inp == "v":
                        drain(proj_v(xt, p))
                    else:
                        drain(proj_qk(xt, inp, p))

            # attn(0) with pair-2 projection as filler
            def proj_b_gen():
                for inp in ("q", "k", "v"):
                    xt = load_xt("b", inp)
                    if inp == "v":
                        yield from proj_v(xt, 2)
                    else:
                        yield from proj_qk(xt, inp, 2)

            f0 = proj_b_gen()
            attention_pair(0, filler=f0)
            drain(f0)

            # attn(1): emit y0 out-projection chunks as each j completes
            # (needs oT[0] full + oT[1] cols through 512*(j+1)).
            def post_j1(j):
                drain(out_proj(y0, (0, 1), range(4 * j, 4 * (j + 1))))

            attention_pair(1, post_j=post_j1)

            # attn(2): same for y1 (pair 2 only).
            def post_j2(j):
                drain(out_proj(y1, (2,), range(4 * j, 4 * (j + 1))))

            attention_pair(2, post_j=post_j2)

    nc.compile()
    return nc


def _get_program():
    if "nc" not in _CACHE:
        _CACHE["nc"] = _build_program()
    return _CACHE["nc"]


def _prep_inputs(query, key, value, Wq, bq, Wk, bk, Wv, bv, Wo, bo):
    """Build the 8 per-core input maps (host-side slicing + bf16 cast)."""
    q = np.asarray(query, np.float32).reshape(8192, 1024).astype(BF16)
    k = np.asarray(key, np.float32).reshape(8192, 1024).astype(BF16)
    v = np.asarray(value, np.float32).reshape(8192, 1024).astype(BF16)
    wq = np.asarray(Wq, np.float32).astype(BF16)
    wk = np.asarray(Wk, np.float32).astype(BF16)
    wv = np.asarray(Wv, np.float32).astype(BF16)
    wo_f = np.asarray(Wo, np.float32).astype(BF16)
    bqf = np.asarray(bq, np.float32)
    bkf = np.asarray(bk, np.float32)

    qT, kT, vT = q.T, k.T, v.T  # [1024, 8192] views
    in_maps = []
    for c in range(8):
        a, sc, b, rc = c % 2, c // 2, c % 4, c // 4
        rows_g0 = slice(2048 * sc, 2048 * (sc + 1))
        rows_g1 = slice(4096 * rc + 1, 4096 * (rc + 1), 2)
        hrows = np.r_[256 * a:256 * a + 256, 512 + 128 * b:512 + 128 * b + 128]
        m = {
            "xa_q": np.ascontiguousarray(qT[:, rows_g0]),
            "xa_k": np.ascontiguousarray(kT[:, rows_g0]),
            "xa_v": np.ascontiguousarray(vT[:, rows_g0]),
            "xb_q": np.ascontiguousarray(qT[:, rows_g1]),
            "xb_k": np.ascontiguousarray(kT[:, rows_g1]),
            "xb_v": np.ascontiguousarray(vT[:, rows_g1]),
            "wq": np.ascontiguousarray(wq[hrows].T),
            "wk": np.ascontiguousarray(wk[hrows].T),
            "wv": np.ascontiguousarray(wv[hrows].T),
            "wo": np.ascontiguousarray(wo_f[:, hrows].T),
            "bq": np.ascontiguousarray(bqf[hrows]).reshape(384, 1),
            "bk": np.ascontiguousarray(bkf[hrows]).reshape(384, 1),
        }
        in_maps.append(m)
    return in_maps


def _combine(results, Wo, bv, bo):
    y = np.zeros((8192, 1024), np.float32)
    for c in range(8):
        sc, rc = c // 2, c // 4
        y[2048 * sc:2048 * (sc + 1)] += np.asarray(
            results[c]["y0"], np.float32)
        y[4096 * rc + 1:4096 * (rc + 1):2] += np.asarray(
            results[c]["y1"], np.float32)
    # bv pass-through: softmax rows sum to 1, so v-bias adds linearly.
    bvf = np.asarray(bv, np.float32)
    if np.any(bvf):
        wof = np.asarray(Wo, np.float32)
        y += bvf[:512] @ wof[:, :512].T                # g0 heads: all rows
        y[1::2] += bvf[512:] @ wof[:, 512:].T          # g1 heads: odd rows
    y += np.asarray(bo, np.float32)
    return y.reshape(1, 8192, 1024)


def kernel(query, key, value, Wq, bq, Wk, bk, Wv, bv, Wo, bo,
           _trace=False, _trace_cores=None):
    from concourse import bass_utils
    nc = _get_program()
    in_maps = _prep_inputs(query, key, value, Wq, bq, Wk, bk, Wv, bv, Wo, bo)
    res = bass_utils.run_bass_kernel_spmd(
        nc, in_maps, core_ids=list(range(8)),
        trace=_trace, trace_cores=_trace_cores)
    _CACHE["last_results"] = res
    return _combine(res.results, Wo, bv, bo)


# revision 10
# speedup vs baseline: 1.1617x; 1.1617x over previous
"""BlockSparseRingMultiheadDilatedAttention Trainium2 kernel (v2).

Problem (hardcoded): B=1, N=8192, E=1024, H=16 heads, D=64.
Two dilated groups: g0 = heads 0-7, seg 2048, dilation 1;
                    g1 = heads 8-15, seg 4096, dilation 2, offset 1 (odd positions).
Causal within each (gathered) segment.

Sharding over 8 cores (uniform SPMD program, per-core data):
  core c: a = c%2, sc = c//2, b = c%4, rc = c//4
    g0: seg sc (rows 2048*sc .. +2048), heads 4a..4a+4   (4 blocks of [2048 x 2048])
    g1: seg rc odd rows (gathered, 2048 rows), heads 8+2b..+2 (2 blocks)
  Host pre-slices inputs (bf16 cast, odd-row gather, weight head slices) so the
  device program is identical on every core.  Host sums the per-core partial
  output projections (disjoint head contributions) and adds bo + the bv
  pass-through term (softmax rows sum to 1, so bv adds linearly after).

v2 device dataflow per core (vs v1 baseline):
  - S^T score matmuls for the 2 heads of a pair are row-tiled (K=64 at
    tile_position (0,0)/(64,0)) and emitted adjacently -> run concurrently.
    s tile [128,1024] = kpos chunk i for BOTH heads (bank-aligned halves).
  - Causal trimming: diagonal chunks only compute/exp/mask columns >= trim.
  - One exp per chunk over both heads via a [128,2,W] strided AP.
  - Single triangle mask constant, applied to the 4 diagonal chunks per
    (j, pair) with a DVE multiply.
  - V projected directly in natural layout (x chunk as stationary operand),
    no PE transposes / per-chunk DVE copies.
  - reciprocal_approx_fast for softmax denominators (5x faster than
    nc.vector.reciprocal).
  - y0/y1 stored bf16 (halves output DMA).
  - Emission interleaves pair-2 projections and output projections into the
    ACT-paced attention phases so the PE stays busy.
"""

import numpy as np
import ml_dtypes

BF16 = ml_dtypes.bfloat16

SEG = 2048          # rows per attention block (both groups, post-gather)
E = 1024            # embedding
NQ = 512            # tq chunk (one PSUM bank of fp32)
NTQ = SEG // NQ     # 4 tq chunks per block
NTK = SEG // 128    # 16 tk chunks per block
ECH = E // 128      # 8 embedding chunks
VSP = 66            # per-head stride in the vn tile (64 v dims + ones + pad)

_CACHE = {}


def _build_program():
    import concourse.bacc as bacc
    import concourse.mybir as mybir
    import concourse.tile as tile

    dt = mybir.dt
    nc = bacc.Bacc("TRN2", target_bir_lowering=False, debug=False,
                   enable_asserts=False)

    # ---- DRAM I/O (uniform across cores; host slices per core) ----
    xs = {}
    for sel in ("a", "b"):      # a = g0 rows, b = g1 gathered odd rows
        for inp in ("q", "k", "v"):
            xs[(sel, inp)] = nc.dram_tensor(
                f"x{sel}_{inp}", [E, SEG], dt.bfloat16, kind="ExternalInput").ap()
    ws = {inp: nc.dram_tensor(f"w{inp}", [E, 384], dt.bfloat16,
                              kind="ExternalInput").ap()
          for inp in ("q", "k", "v")}
    wo = nc.dram_tensor("wo", [384, E], dt.bfloat16, kind="ExternalInput").ap()
    bs = {inp: nc.dram_tensor(f"b{inp}", [384, 1], dt.float32,
                              kind="ExternalInput").ap()
          for inp in ("q", "k")}
    y0 = nc.dram_tensor("y0", [SEG, E], dt.bfloat16, kind="ExternalOutput").ap()
    y1 = nc.dram_tensor("y1", [SEG, E], dt.bfloat16, kind="ExternalOutput").ap()

    with tile.TileContext(nc) as tc:
        from contextlib import ExitStack
        with ExitStack() as ctx:
            const = ctx.enter_context(tc.tile_pool(name="const", bufs=1))
            wpool = ctx.enter_context(tc.tile_pool(name="wpool", bufs=1))
            xtp = ctx.enter_context(tc.tile_pool(name="xtp", bufs=2))
            qkt = ctx.enter_context(tc.tile_pool(name="qkt", bufs=1))
            vnat = ctx.enter_context(tc.tile_pool(name="vnat", bufs=1))
            otp = ctx.enter_context(tc.tile_pool(name="otp", bufs=1))
            ptp = ctx.enter_context(tc.tile_pool(name="ptp", bufs=3))
            smallp = ctx.enter_context(tc.tile_pool(name="smallp", bufs=4))
            ypool = ctx.enter_context(tc.tile_pool(name="ypool", bufs=4))
            ps_mm = ctx.enter_context(
                tc.tile_pool(name="ps_mm", bufs=3, space="PSUM"))
            ps_acc = ctx.enter_context(
                tc.tile_pool(name="ps_acc", bufs=2, space="PSUM"))

            # ---- constants: triangle mask (shared by both heads/halves) ----
            # maskC[p, 512*h + w] = 1.0 if w >= p else 0.0
            maskC = const.tile([128, 2 * NQ], dt.bfloat16, tag="maskC")
            nc.gpsimd.memset(maskC, 1.0)
            nc.gpsimd.affine_select(
                out=maskC.rearrange("p (h w) -> p h w", h=2),
                in_=maskC.rearrange("p (h w) -> p h w", h=2),
                compare_op=mybir.AluOpType.is_ge,
                fill=0.0, base=0, pattern=[[0, 2], [1, NQ]],
                channel_multiplier=-1)

            # ---- weights (host-pre-transposed; plain DMA loads) ----
            # wTa[inp]: [128 e, ECH*384]; chunk ec pair p at 384*ec + 128*p
            wTa = {}
            for inp in ("q", "k", "v"):
                t = wpool.tile([128, 384 * ECH], dt.bfloat16,
                               tag=f"wT_{inp}", name=f"wT_{inp}")
                for ec in range(ECH):
                    nc.sync.dma_start(
                        out=t[:, 384 * ec:384 * (ec + 1)],
                        in_=ws[inp][128 * ec:128 * (ec + 1), :])
                wTa[inp] = t
            # wT[inp][p]: 3D view [128 e, ECH, 128 d]; chunk ec = [:, ec, :]
            wT = {inp: [wTa[inp].rearrange("p (ec x) -> p ec x", x=384)
                        [:, :, 128 * p:128 * (p + 1)]
                        for p in range(3)] for inp in ("q", "k", "v")}
            # woT[p]: [128 (2 heads d), 1024 j]
            woT = []
            for p in range(3):
                t = wpool.tile([128, E], dt.bfloat16, tag=f"woT_{p}")
                nc.sync.dma_start(out=t, in_=wo[128 * p:128 * (p + 1), :])
                woT.append(t)
            # biases -> SBUF [128,1] per (inp, pair), q/k only
            bsb = {}
            for inp in ("q", "k"):
                for p in range(3):
                    t = wpool.tile([128, 1], dt.float32, tag=f"b_{inp}_{p}")
                    nc.sync.dma_start(
                        out=t, in_=bs[inp][128 * p:128 * (p + 1), :])
                    bsb[(inp, p)] = t

            # ---- persistent per-pair activations ----
            qT = [qkt.tile([128, SEG], dt.bfloat16, tag=f"qT{p}", name=f"qT{p}")
                  for p in range(3)]
            kT = [qkt.tile([128, SEG], dt.bfloat16, tag=f"kT{p}", name=f"kT{p}")
                  for p in range(3)]
            # V natural: per pair [128, NTK*2*VSP]; chunk i head h lhsT (65
            # cols: 64 v dims + ones) = [:, 2*VSP*i + VSP*h : +65]
            vn = [vnat.tile([128, NTK * 2 * VSP], dt.bfloat16,
                            tag=f"vn{p}", name=f"vn{p}")
                  for p in range(3)]
            oT = [otp.tile([128, SEG], dt.bfloat16, tag=f"oT{p}", name=f"oT{p}")
                  for p in range(3)]

            # ---- projection emitters (generators yielding PE quanta) ----
            def load_xt(sel, inp):
                xt = xtp.tile([128, ECH * SEG], dt.bfloat16, tag="xt")
                for ec in range(ECH):
                    nc.sync.dma_start(
                        out=xt[:, SEG * ec:SEG * (ec + 1)],
                        in_=xs[(sel, inp)][128 * ec:128 * (ec + 1), :])
                return xt

            def proj_qk(xt, inp, p):
                """Emit projection of one input tensor into qT/kT[p]."""
                dst = (qT if inp == "q" else kT)[p]
                for t2 in range(NTQ // 2):
                    acc = ps_mm.tile([128, 2 * NQ], dt.float32, tag="mm")
                    for half in range(2):
                        for ec in range(ECH):
                            nc.tensor.matmul(
                                acc[:, NQ * half:NQ * (half + 1)],
                                wT[inp][p][:, ec, :],
                                xt[:, SEG * ec + NQ * (2 * t2 + half):
                                   SEG * ec + NQ * (2 * t2 + half + 1)],
                                start=(ec == 0), stop=(ec == ECH - 1))
                        yield
                    nc.scalar.activation(
                        dst[:, 2 * NQ * t2:2 * NQ * (t2 + 1)], acc,
                        mybir.ActivationFunctionType.Identity,
                        bias=bsb[(inp, p)], scale=1.0)

            def proj_v(xt, p):
                """Emit V-natural projection into vn[p] (+ ones columns)."""
                # rank-3 views only (keep APs simple for lowering)
                vv = vn[p].rearrange("p (kh y) -> p kh y", y=VSP)
                nc.gpsimd.memset(vv[:, :, 64:65], 1.0)
                vk = vn[p].rearrange("p (k y) -> p k y", y=2 * VSP)
                for g in range(2):          # 8 kpos chunks per accv tile
                    accv = ps_mm.tile([128, 2 * NQ], dt.float32, tag="mm")
                    for kbs in range(8):
                        kb = 8 * g + kbs
                        for ec in range(ECH):
                            nc.tensor.matmul(
                                accv[:, 128 * kbs:128 * (kbs + 1)],
                                xt[:, SEG * ec + 128 * kb:
                                   SEG * ec + 128 * (kb + 1)],
                                wT["v"][p][:, ec, :],
                                start=(ec == 0), stop=(ec == ECH - 1))
                        if kbs % 2 == 1:
                            yield
                    av = accv.rearrange("p (k y) -> p k y", y=128)
                    for h in range(2):
                        nc.vector.tensor_copy(
                            vk[:, 8 * g:8 * (g + 1),
                               VSP * h:VSP * h + 64],
                            av[:, :, 64 * h:64 * h + 64])
                    yield

            def out_proj(ydram, pairs, ms):
                """Emit output-projection chunks for row-chunks in ms."""
                for m in ms:
                    accy = ps_mm.tile([128, 2 * NQ], dt.float32, tag="mm",
                                      name="accy")
                    for jc in range(2):
                        for idx, p in enumerate(pairs):
                            nc.tensor.matmul(
                                accy[:, NQ * jc:NQ * (jc + 1)],
                                oT[p][:, 128 * m:128 * (m + 1)],
                                woT[p][:, NQ * jc:NQ * (jc + 1)],
                                start=(idx == 0), stop=(idx == len(pairs) - 1))
                    ysb = ypool.tile([128, 2 * NQ], dt.bfloat16, tag="ysb")
                    nc.vector.tensor_copy(ysb, accy)
                    nc.sync.dma_start(
                        out=ydram[128 * m:128 * (m + 1), :], in_=ysb)
                    yield

            def drain(gen):
                if gen is not None:
                    for _ in gen:
                        pass

            # ---- attention for one pair, with filler interleaving ----
            def attention_pair(p, filler=None, post_j=None):
                def pull():
                    if filler is not None:
                        next(filler, None)

                for j in range(NTQ):
                    nchunks = 4 * (j + 1)
                    acc = [ps_acc.tile([128, NQ], dt.float32, tag="acc",
                                       name=f"acc{h}") for h in range(2)]
                    pend = []   # (i, trim, pt) awaiting O emission

                    def s_chunk(i):
                        """Row-tiled S matmuls + exp (+ mask) for chunk i."""
                        trim = max(0, 128 * i - NQ * j)
                        s = ps_mm.tile([128, 2 * NQ], dt.float32,
                                       tag="mm", name="s")
                        for h in range(2):
                            hp = 64 * h
                            nc.tensor.matmul(
                                s[:, NQ * h + trim:NQ * (h + 1)],
                                kT[p][hp:hp + 64, 128 * i:128 * (i + 1)],
                                qT[p][hp:hp + 64,
                                      NQ * j + trim:NQ * (j + 1)],
                                start=True, stop=True)
                        pt = ptp.tile([128, 2 * NQ], dt.bfloat16,
                                      tag="pt", name="pt")
                        sv = s.rearrange("p (h w) -> p h w", h=2)
                        pv = pt.rearrange("p (h w) -> p h w", h=2)
                        nc.scalar.activation(
                            pv[:, :, trim:NQ], sv[:, :, trim:NQ],
                            mybir.ActivationFunctionType.Exp,
                            bias=0.0, scale=0.125)
                        if trim > 0 or i == 4 * j:   # diagonal chunk
                            mv = maskC.rearrange("p (h w) -> p h w", h=2)
                            nc.vector.tensor_mul(
                                pv[:, :, trim:NQ], pv[:, :, trim:NQ],
                                mv[:, :, 0:NQ - trim])
                        return trim, pt

                    def o_chunk(i, trim, pt):
                        pv = pt.rearrange("p (h w) -> p h w", h=2)
                        for h in range(2):
                            nc.tensor.matmul(
                                acc[h][0:65, trim:NQ],
                                vn[p][:, 2 * VSP * i + VSP * h:
                                      2 * VSP * i + VSP * h + 65],
                                pv[:, h, trim:NQ],
                                start=(i == 0), stop=(i == nchunks - 1),
                                skip_group_check=True)

                    for i in range(nchunks):
                        pend.append((i, *s_chunk(i)))
                        if len(pend) > 1:
                            o_chunk(*pend.pop(0))
                            pull()
                    o_chunk(*pend.pop(0))
                    pull()

                    # normalize: r = 1/d, broadcast, multiply into oT
                    for h in range(2):
                        hp = 64 * h
                        dj = smallp.tile([1, NQ], dt.float32, tag="dj",
                                         name="dj")
                        nc.vector.tensor_copy(dj, acc[h][64:65, :])
                        rj = smallp.tile([1, NQ], dt.float32, tag="rj",
                                         name="rj")
                        nc.vector.reciprocal_approx_fast(out=rj, in_=dj)
                        rb = smallp.tile([64, NQ], dt.float32, tag="rb",
                                         name="rb")
                        nc.gpsimd.partition_broadcast(rb, rj)
                        nc.vector.tensor_mul(
                            oT[p][hp:hp + 64, NQ * j:NQ * (j + 1)],
                            acc[h][0:64, :], rb)
                    if post_j is not None:
                        post_j(j)
                    pull()

            # ================= emission schedule =================
            # Head: load + project xa (pairs 0,1) densely.
            for inp in ("q", "k", "v"):
                xt = load_xt("a", inp)
                for p in (0, 1):
                    if inp == "v":
                        drain(proj_v(xt, p))
                    else:
                        drain(proj_qk(xt, inp, p))

            # attn(0) with pair-2 projection as filler
            def proj_b_gen():
                for inp in ("q", "k", "v"):
                    xt = load_xt("b", inp)
                    if inp == "v":
                        yield from proj_v(xt, 2)
                    else:
                        yield from proj_qk(xt, inp, 2)

            f0 = proj_b_gen()
            attention_pair(0, filler=f0)
            drain(f0)

            # attn(1): emit y0 out-projection chunks as each j completes
            # (needs oT[0] full + oT[1] cols through 512*(j+1)).
            def post_j1(j):
                drain(out_proj(y0, (0, 1), range(4 * j, 4 * (j + 1))))

            attention_pair(1, post_j=post_j1)

            # attn(2): same for y1 (pair 2 only).
            def post_j2(j):
                drain(out_proj(y1, (2,), range(4 * j, 4 * (j + 1))))

            attention_pair(2, post_j=post_j2)

    nc.compile()
    return nc


def _get_program():
    if "nc" not in _CACHE:
        _CACHE["nc"] = _build_program()
    return _CACHE["nc"]


def _prep_inputs(query, key, value, Wq, bq, Wk, bk, Wv, bv, Wo, bo):
    """Build the 8 per-core input maps (host-side slicing + bf16 cast)."""
    q = np.asarray(query, np.float32).reshape(8192, 1024).astype(BF16)
    k = np.asarray(key, np.float32).reshape(8192, 1024).astype(BF16)
    v = np.asarray(value, np.float32).reshape(8192, 1024).astype(BF16)
    wq = np.asarray(Wq, np.float32).astype(BF16)
    wk = np.asarray(Wk, np.float32).astype(BF16)
    wv = np.asarray(Wv, np.float32).astype(BF16)
    wo_f = np.asarray(Wo, np.float32).astype(BF16)
    bqf = np.asarray(bq, np.float32)
    bkf = np.asarray(bk, np.float32)

    qT, kT, vT = q.T, k.T, v.T  # [1024, 8192] views
    in_maps = []
    for c in range(8):
        a, sc, b, rc = c % 2, c // 2, c % 4, c // 4
        rows_g0 = slice(2048 * sc, 2048 * (sc + 1))
        rows_g1 = slice(4096 * rc + 1, 4096 * (rc + 1), 2)
        hrows = np.r_[256 * a:256 * a + 256, 512 + 128 * b:512 + 128 * b + 128]
        m = {
            "xa_q": np.ascontiguousarray(qT[:, rows_g0]),
            "xa_k": np.ascontiguousarray(kT[:, rows_g0]),
            "xa_v": np.ascontiguousarray(vT[:, rows_g0]),
            "xb_q": np.ascontiguousarray(qT[:, rows_g1]),
            "xb_k": np.ascontiguousarray(kT[:, rows_g1]),
            "xb_v": np.ascontiguousarray(vT[:, rows_g1]),
            "wq": np.ascontiguousarray(wq[hrows].T),
            "wk": np.ascontiguousarray(wk[hrows].T),
            "wv": np.ascontiguousarray(wv[hrows].T),
            "wo": np.ascontiguousarray(wo_f[:, hrows].T),
            "bq": np.ascontiguousarray(bqf[hrows]).reshape(384, 1),
            "bk": np.ascontiguousarray(bkf[hrows]).reshape(384, 1),
        }
        in_maps.append(m)
    return in_maps


def _combine(results, Wo, bv, bo):
    y = np.zeros((8192, 1024), np.float32)
    for c in range(8):
        sc, rc = c // 2, c // 4
        y[2048 * sc:2048 * (sc + 1)] += np.asarray(
            results[c]["y0"], np.float32)
        y[4096 * rc + 1:4096 * (rc + 1):2] += np.asarray(
            results[c]["y1"], np.float32)
    # bv pass-through: softmax rows sum to 1, so v-bias adds linearly.
    bvf = np.asarray(bv, np.float32)
    if np.any(bvf):
        wof = np.asarray(Wo, np.float32)
        y += bvf[:512] @ wof[:, :512].T                # g0 heads: all rows
        y[1::2] += bvf[512:] @ wof[:, 512:].T          # g1 heads: odd rows
    y += np.asarray(bo, np.float32)
    return y.reshape(1, 8192, 1024)


def kernel(query, key, value, Wq, bq, Wk, bk, Wv, bv, Wo, bo,
           _trace=False, _trace_cores=None):
    from concourse import bass_utils
    nc = _get_program()
    in_maps = _prep_inputs(query, key, value, Wq, bq, Wk, bk, Wv, bv, Wo, bo)
    res = bass_utils.run_bass_kernel_spmd(
        nc, in_maps, core_ids=list(range(8)),
        trace=_trace, trace_cores=_trace_cores)
    _CACHE["last_results"] = res
    return _combine(res.results, Wo, bv, bo)


# revision 18
# speedup vs baseline: 1.2169x; 1.0475x over previous
"""BlockSparseRingMultiheadDilatedAttention Trainium2 kernel (v2).

Problem (hardcoded): B=1, N=8192, E=1024, H=16 heads, D=64.
Two dilated groups: g0 = heads 0-7, seg 2048, dilation 1;
                    g1 = heads 8-15, seg 4096, dilation 2, offset 1 (odd positions).
Causal within each (gathered) segment.

Sharding over 8 cores (uniform SPMD program, per-core data):
  core c: a = c%2, sc = c//2, b = c%4, rc = c//4
    g0: seg sc (rows 2048*sc .. +2048), heads 4a..4a+4   (4 blocks of [2048 x 2048])
    g1: seg rc odd rows (gathered, 2048 rows), heads 8+2b..+2 (2 blocks)
  Host pre-slices inputs (bf16 cast, odd-row gather, weight head slices) so the
  device program is identical on every core.  Host sums the per-core partial
  output projections (disjoint head contributions) and adds bo + the bv
  pass-through term (softmax rows sum to 1, so bv adds linearly after).

v2 device dataflow per core (vs v1 baseline):
  - S^T score matmuls for the 2 heads of a pair are row-tiled (K=64 at
    tile_position (0,0)/(64,0)) and emitted adjacently -> run concurrently.
    s tile [128,1024] = kpos chunk i for BOTH heads (bank-aligned halves).
  - Causal trimming: diagonal chunks only compute/exp/mask columns >= trim.
  - One exp per chunk over both heads via a [128,2,W] strided AP.
  - Single triangle mask constant, applied to the 4 diagonal chunks per
    (j, pair) with a DVE multiply.
  - V projected directly in natural layout (x chunk as stationary operand),
    no PE transposes / per-chunk DVE copies.
  - reciprocal_approx_fast for softmax denominators (5x faster than
    nc.vector.reciprocal).
  - y0/y1 stored bf16 (halves output DMA).
  - Emission interleaves pair-2 projections and output projections into the
    ACT-paced attention phases so the PE stays busy.
"""

import numpy as np
import ml_dtypes

BF16 = ml_dtypes.bfloat16

SEG = 2048          # rows per attention block (both groups, post-gather)
E = 1024            # embedding
NQ = 512            # tq chunk (one PSUM bank of fp32)
NTQ = SEG // NQ     # 4 tq chunks per block
NTK = SEG // 128    # 16 tk chunks per block
ECH = E // 128      # 8 embedding chunks
VSP = 66            # per-head stride in the vn tile (64 v dims + ones + pad)

_CACHE = {}


def _build_program():
    import concourse.bacc as bacc
    import concourse.mybir as mybir
    import concourse.tile as tile

    dt = mybir.dt
    nc = bacc.Bacc("TRN2", target_bir_lowering=False, debug=False,
                   enable_asserts=False)

    # ---- DRAM I/O (uniform across cores; host slices per core) ----
    xs = {}
    for sel in ("a", "b"):      # a = g0 rows, b = g1 gathered odd rows
        for inp in ("q", "k", "v"):
            xs[(sel, inp)] = nc.dram_tensor(
                f"x{sel}_{inp}", [E, SEG], dt.bfloat16, kind="ExternalInput").ap()
    ws = {inp: nc.dram_tensor(f"w{inp}", [E, 384], dt.bfloat16,
                              kind="ExternalInput").ap()
          for inp in ("q", "k", "v")}
    wo = nc.dram_tensor("wo", [384, E], dt.bfloat16, kind="ExternalInput").ap()
    bs = {inp: nc.dram_tensor(f"b{inp}", [384, 1], dt.float32,
                              kind="ExternalInput").ap()
          for inp in ("q", "k")}
    y0 = nc.dram_tensor("y0", [SEG, E], dt.bfloat16, kind="ExternalOutput").ap()
    y1 = nc.dram_tensor("y1", [SEG, E], dt.bfloat16, kind="ExternalOutput").ap()

    with tile.TileContext(nc) as tc:
        from contextlib import ExitStack
        with ExitStack() as ctx:
            const = ctx.enter_context(tc.tile_pool(name="const", bufs=1))
            wpool = ctx.enter_context(tc.tile_pool(name="wpool", bufs=1))
            xtp = ctx.enter_context(tc.tile_pool(name="xtp", bufs=3))
            qkt = ctx.enter_context(tc.tile_pool(name="qkt", bufs=1))
            vnat = ctx.enter_context(tc.tile_pool(name="vnat", bufs=1))
            otp = ctx.enter_context(tc.tile_pool(name="otp", bufs=1))
            ptp = ctx.enter_context(tc.tile_pool(name="ptp", bufs=3))
            smallp = ctx.enter_context(tc.tile_pool(name="smallp", bufs=4))
            ypool = ctx.enter_context(tc.tile_pool(name="ypool", bufs=3))
            ps_mm = ctx.enter_context(
                tc.tile_pool(name="ps_mm", bufs=3, space="PSUM"))
            ps_acc = ctx.enter_context(
                tc.tile_pool(name="ps_acc", bufs=2, space="PSUM"))

            # ---- constants: triangle mask (shared by both heads/halves) ----
            # maskC[p, 512*h + w] = 1.0 if w >= p else 0.0
            maskC = const.tile([128, 2 * NQ], dt.bfloat16, tag="maskC")
            nc.gpsimd.memset(maskC, 1.0)
            nc.gpsimd.affine_select(
                out=maskC.rearrange("p (h w) -> p h w", h=2),
                in_=maskC.rearrange("p (h w) -> p h w", h=2),
                compare_op=mybir.AluOpType.is_ge,
                fill=0.0, base=0, pattern=[[0, 2], [1, NQ]],
                channel_multiplier=-1)

            # ---- weights (host-pre-transposed; plain DMA loads) ----
            # wTa[inp]: [128 e, ECH*384]; chunk ec pair p at 384*ec + 128*p
            wTa = {}
            for inp in ("q", "k", "v"):
                t = wpool.tile([128, 384 * ECH], dt.bfloat16,
                               tag=f"wT_{inp}", name=f"wT_{inp}")
                for ec in range(ECH):
                    nc.sync.dma_start(
                        out=t[:, 384 * ec:384 * (ec + 1)],
                        in_=ws[inp][128 * ec:128 * (ec + 1), :])
                wTa[inp] = t
            # wT[inp][p]: 3D view [128 e, ECH, 128 d]; chunk ec = [:, ec, :]
            wT = {inp: [wTa[inp].rearrange("p (ec x) -> p ec x", x=384)
                        [:, :, 128 * p:128 * (p + 1)]
                        for p in range(3)] for inp in ("q", "k", "v")}
            # woT[p]: [128 (2 heads d), 1024 j]
            woT = []
            for p in range(3):
                t = wpool.tile([128, E], dt.bfloat16, tag=f"woT_{p}")
                nc.sync.dma_start(out=t, in_=wo[128 * p:128 * (p + 1), :])
                woT.append(t)
            # biases -> SBUF [128,1] per (inp, pair), q/k only
            bsb = {}
            for inp in ("q", "k"):
                for p in range(3):
                    t = wpool.tile([128, 1], dt.float32, tag=f"b_{inp}_{p}")
                    nc.sync.dma_start(
                        out=t, in_=bs[inp][128 * p:128 * (p + 1), :])
                    bsb[(inp, p)] = t

            # ---- persistent per-pair activations ----
            qT = [qkt.tile([128, SEG], dt.bfloat16, tag=f"qT{p}", name=f"qT{p}")
                  for p in range(3)]
            kT = [qkt.tile([128, SEG], dt.bfloat16, tag=f"kT{p}", name=f"kT{p}")
                  for p in range(3)]
            # V natural: per pair [128, NTK*2*VSP]; chunk i head h lhsT (65
            # cols: 64 v dims + ones) = [:, 2*VSP*i + VSP*h : +65]
            vn = [vnat.tile([128, NTK * 2 * VSP], dt.bfloat16,
                            tag=f"vn{p}", name=f"vn{p}")
                  for p in range(3)]
            oT = [otp.tile([128, SEG], dt.bfloat16, tag=f"oT{p}", name=f"oT{p}")
                  for p in range(3)]

            # ---- projection emitters (generators yielding PE quanta) ----
            def load_xt(sel, inp):
                xt = xtp.tile([128, ECH * SEG], dt.bfloat16, tag="xt")
                for ec in range(ECH):
                    for hf in range(2):     # 16 DMAs -> spread over all queues
                        nc.sync.dma_start(
                            out=xt[:, SEG * ec + 1024 * hf:
                                   SEG * ec + 1024 * (hf + 1)],
                            in_=xs[(sel, inp)][128 * ec:128 * (ec + 1),
                                               1024 * hf:1024 * (hf + 1)])
                return xt

            def proj_qk(xt, inp, p):
                """Emit projection of one input tensor into qT/kT[p]."""
                dst = (qT if inp == "q" else kT)[p]
                for t2 in range(NTQ // 2):
                    acc = ps_mm.tile([128, 2 * NQ], dt.float32, tag="mm")
                    for half in range(2):
                        for ec in range(ECH):
                            nc.tensor.matmul(
                                acc[:, NQ * half:NQ * (half + 1)],
                                wT[inp][p][:, ec, :],
                                xt[:, SEG * ec + NQ * (2 * t2 + half):
                                   SEG * ec + NQ * (2 * t2 + half + 1)],
                                start=(ec == 0), stop=(ec == ECH - 1))
                        yield
                    nc.scalar.activation(
                        dst[:, 2 * NQ * t2:2 * NQ * (t2 + 1)], acc,
                        mybir.ActivationFunctionType.Identity,
                        bias=bsb[(inp, p)], scale=1.0)

            def proj_v(xt, p):
                """Emit V-natural projection into vn[p] (+ ones columns)."""
                # rank-3 views only (keep APs simple for lowering)
                vv = vn[p].rearrange("p (kh y) -> p kh y", y=VSP)
                nc.gpsimd.memset(vv[:, :, 64:65], 1.0)
                vk = vn[p].rearrange("p (k y) -> p k y", y=2 * VSP)
                for g in range(2):          # 8 kpos chunks per accv tile
                    accv = ps_mm.tile([128, 2 * NQ], dt.float32, tag="mm")
                    for kbs in range(8):
                        kb = 8 * g + kbs
                        for ec in range(ECH):
                            nc.tensor.matmul(
                                accv[:, 128 * kbs:128 * (kbs + 1)],
                                xt[:, SEG * ec + 128 * kb:
                                   SEG * ec + 128 * (kb + 1)],
                                wT["v"][p][:, ec, :],
                                start=(ec == 0), stop=(ec == ECH - 1))
                        if kbs % 2 == 1:
                            yield
                    av = accv.rearrange("p (k y) -> p k y", y=128)
                    for h in range(2):
                        nc.vector.tensor_copy(
                            vk[:, 8 * g:8 * (g + 1),
                               VSP * h:VSP * h + 64],
                            av[:, :, 64 * h:64 * h + 64])
                    yield

            def out_proj(ydram, pairs, ms):
                """Emit output-projection chunks for row-chunks in ms."""
                for m in ms:
                    accy = ps_mm.tile([128, 2 * NQ], dt.float32, tag="mm",
                                      name="accy")
                    for jc in range(2):
                        for idx, p in enumerate(pairs):
                            nc.tensor.matmul(
                                accy[:, NQ * jc:NQ * (jc + 1)],
                                oT[p][:, 128 * m:128 * (m + 1)],
                                woT[p][:, NQ * jc:NQ * (jc + 1)],
                                start=(idx == 0), stop=(idx == len(pairs) - 1))
                    ysb = ypool.tile([128, 2 * NQ], dt.bfloat16, tag="ysb")
                    nc.vector.tensor_copy(ysb, accy)
                    nc.sync.dma_start(
                        out=ydram[128 * m:128 * (m + 1), :], in_=ysb)
                    yield

            def drain(gen):
                if gen is not None:
                    for _ in gen:
                        pass

            # ---- attention for one pair, with filler interleaving ----
            def attention_pair(p, filler=None, post_j=None):
                def pull():
                    if filler is not None:
                        next(filler, None)

                for j in range(NTQ):
                    nchunks = 4 * (j + 1)
                    acc = [ps_acc.tile([128, NQ], dt.float32, tag="acc",
                                       name=f"acc{h}") for h in range(2)]
                    pend = []   # (i, trim, pt) awaiting O emission

                    def s_chunk(i):
                        """Row-tiled S matmuls + exp (+ mask) for chunk i."""
                        trim = max(0, 128 * i - NQ * j)
                        s = ps_mm.tile([128, 2 * NQ], dt.float32,
                                       tag="mm", name="s")
                        for h in range(2):
                            hp = 64 * h
                            nc.tensor.matmul(
                                s[:, NQ * h + trim:NQ * (h + 1)],
                                kT[p][hp:hp + 64, 128 * i:128 * (i + 1)],
                                qT[p][hp:hp + 64,
                                      NQ * j + trim:NQ * (j + 1)],
                                start=True, stop=True)
                        pt = ptp.tile([128, 2 * NQ], dt.bfloat16,
                                      tag="pt", name="pt")
                        sv = s.rearrange("p (h w) -> p h w", h=2)
                        pv = pt.rearrange("p (h w) -> p h w", h=2)
                        nc.scalar.activation(
                            pv[:, :, trim:NQ], sv[:, :, trim:NQ],
                            mybir.ActivationFunctionType.Exp,
                            bias=0.0, scale=0.125)
                        if trim > 0 or i == 4 * j:   # diagonal chunk
                            mv = maskC.rearrange("p (h w) -> p h w", h=2)
                            nc.vector.tensor_mul(
                                pv[:, :, trim:NQ], pv[:, :, trim:NQ],
                                mv[:, :, 0:NQ - trim])
                        return trim, pt

                    def o_chunk(i, trim, pt):
                        pv = pt.rearrange("p (h w) -> p h w", h=2)
                        for h in range(2):
                            nc.tensor.matmul(
                                acc[h][0:65, trim:NQ],
                                vn[p][:, 2 * VSP * i + VSP * h:
                                      2 * VSP * i + VSP * h + 65],
                                pv[:, h, trim:NQ],
                                start=(i == 0), stop=(i == nchunks - 1),
                                skip_group_check=True)

                    for i in range(nchunks):
                        pend.append((i, *s_chunk(i)))
                        if len(pend) > 1:
                            o_chunk(*pend.pop(0))
                            pull()
                    o_chunk(*pend.pop(0))
                    pull()

                    # normalize: copy acc out early (frees the psum bank),
                    # then 1/d + broadcast + in-place scale off that path.
                    for h in range(2):
                        hp = 64 * h
                        ov = oT[p][hp:hp + 64, NQ * j:NQ * (j + 1)]
                        dj = smallp.tile([1, NQ], dt.float32, tag="dj",
                                         name="dj")
                        nc.vector.tensor_copy(dj, acc[h][64:65, :])
                        nc.vector.tensor_copy(ov, acc[h][0:64, :])
                        rj = smallp.tile([1, NQ], dt.float32, tag="rj",
                                         name="rj")
                        nc.vector.reciprocal_approx_fast(out=rj, in_=dj)
                        rbb = smallp.tile([128, NQ], dt.float32, tag="rb",
                                          name="rb")
                        nc.gpsimd.partition_broadcast(rbb, rj)
                        nc.vector.tensor_mul(ov, ov, rbb[hp:hp + 64, :])
                    if post_j is not None:
                        post_j(j)
                    pull()

            # ================= emission schedule =================
            # Head: kick all xa loads upfront, then project pairs 0,1 densely.
            xts = {inp: load_xt("a", inp) for inp in ("q", "k", "v")}
            for inp in ("q", "k", "v"):
                for p in (0, 1):
                    if inp == "v":
                        drain(proj_v(xts[inp], p))
                    else:
                        drain(proj_qk(xts[inp], inp, p))

            # attn(0) with pair-2 projection as filler
            def proj_b_gen():
                for inp in ("q", "k", "v"):
                    xt = load_xt("b", inp)
                    if inp == "v":
                        yield from proj_v(xt, 2)
                    else:
                        yield from proj_qk(xt, inp, 2)

            f0 = proj_b_gen()
            attention_pair(0, filler=f0)
            drain(f0)

            # attn(1): emit y0 out-projection chunks delayed by one j so the
            # PE stream never waits on the just-finished normalize chain.
            def post_j1(j):
                if j >= 1:
                    drain(out_proj(y0, (0, 1), range(4 * (j - 1), 4 * j)))

            attention_pair(1, post_j=post_j1)

            # attn(2): leftover y0 chunks as lead-in filler, y1 delayed by
            # one j, last y1 chunks as the tail.
            f2 = out_proj(y0, (0, 1), range(12, 16))

            def post_j2(j):
                if j >= 1:
                    drain(out_proj(y1, (2,), range(4 * (j - 1), 4 * j)))

            attention_pair(2, filler=f2, post_j=post_j2)
            drain(f2)
            drain(out_proj(y1, (2,), range(12, 16)))

    nc.compile()
    return nc


def _get_program():
    if "nc" not in _CACHE:
        _CACHE["nc"] = _build_program()
    return _CACHE["nc"]


def _prep_inputs(query, key, value, Wq, bq, Wk, bk, Wv, bv, Wo, bo):
    """Build the 8 per-core input maps (host-side slicing + bf16 cast)."""
    q = np.asarray(query, np.float32).reshape(8192, 1024).astype(BF16)
    k = np.asarray(key, np.float32).reshape(8192, 1024).astype(BF16)
    v = np.asarray(value, np.float32).reshape(8192, 1024).astype(BF16)
    wq = np.asarray(Wq, np.float32).astype(BF16)
    wk = np.asarray(Wk, np.float32).astype(BF16)
    wv = np.asarray(Wv, np.float32).astype(BF16)
    wo_f = np.asarray(Wo, np.float32).astype(BF16)
    bqf = np.asarray(bq, np.float32)
    bkf = np.asarray(bk, np.float32)

    qT, kT, vT = q.T, k.T, v.T  # [1024, 8192] views
    in_maps = []
    for c in range(8):
        a, sc, b, rc = c % 2, c // 2, c % 4, c // 4
        rows_g0 = slice(2048 * sc, 2048 * (sc + 1))
        rows_g1 = slice(4096 * rc + 1, 4096 * (rc + 1), 2)
        hrows = np.r_[256 * a:256 * a + 256, 512 + 128 * b:512 + 128 * b + 128]
        m = {
            "xa_q": np.ascontiguousarray(qT[:, rows_g0]),
            "xa_k": np.ascontiguousarray(kT[:, rows_g0]),
            "xa_v": np.ascontiguousarray(vT[:, rows_g0]),
            "xb_q": np.ascontiguousarray(qT[:, rows_g1]),
            "xb_k": np.ascontiguousarray(kT[:, rows_g1]),
            "xb_v": np.ascontiguousarray(vT[:, rows_g1]),
            "wq": np.ascontiguousarray(wq[hrows].T),
            "wk": np.ascontiguousarray(wk[hrows].T),
            "wv": np.ascontiguousarray(wv[hrows].T),
            "wo": np.ascontiguousarray(wo_f[:, hrows].T),
            "bq": np.ascontiguousarray(bqf[hrows]).reshape(384, 1),
            "bk": np.ascontiguousarray(bkf[hrows]).reshape(384, 1),
        }
        in_maps.append(m)
    return in_maps


def _combine(results, Wo, bv, bo):
    y = np.zeros((8192, 1024), np.float32)
    for c in range(8):
        sc, rc = c // 2, c // 4
        y[2048 * sc:2048 * (sc + 1)] += np.asarray(
            results[c]["y0"], np.float32)
        y[4096 * rc + 1:4096 * (rc + 1):2] += np.asarray(
            results[c]["y1"], np.float32)
    # bv pass-through: softmax rows sum to 1, so v-bias adds linearly.
    bvf = np.asarray(bv, np.float32)
    if np.any(bvf):
        wof = np.asarray(Wo, np.float32)
        y += bvf[:512] @ wof[:, :512].T                # g0 heads: all rows
        y[1::2] += bvf[512:] @ wof[:, 512:].T          # g1 heads: odd rows
    y += np.asarray(bo, np.float32)
    return y.reshape(1, 8192, 1024)


def kernel(query, key, value, Wq, bq, Wk, bk, Wv, bv, Wo, bo,
           _trace=False, _trace_cores=None):
    from concourse import bass_utils
    nc = _get_program()
    in_maps = _prep_inputs(query, key, value, Wq, bq, Wk, bk, Wv, bv, Wo, bo)
    res = bass_utils.run_bass_kernel_spmd(
        nc, in_maps, core_ids=list(range(8)),
        trace=_trace, trace_cores=_trace_cores)
    _CACHE["last_results"] = res
    return _combine(res.results, Wo, bv, bo)
